# revision 1
# baseline (speedup 1.0000x reference)
"""Trainium2 Bass kernel for nn_CoupleLoss (retrieval_knn).

Reference computation:
    protos = id_prototypes.at[label].set(teachor_ftr)          # scatter
    gi     = protos[idH[label, :K]]                            # [B, K, D] gather
    loss   = mean(relu(einsum('bkd,bd->bk', gi, ftr - teachor_ftr) - MARGIN))

Key identity: smrs - tmrs = gi . (ftr - teachor_ftr), so only one dot per
(b, k) pair is needed against delta = ftr - teachor_ftr.

Distribution (8 cores): data-parallel over the batch (64 samples/core).
The host performs the index routing (applies the tiny teacher scatter and
resolves each core's 6400 = 64*100 prototype row ids) and ships each core
its row shard in compute order, d-major — measured on this part, on-device
row-gather descriptor generation (SWDGE/Q7, indirect DMA and the dma_gather
ucode alike) tops out at ~8 ns/row (~125 GB/s for 1 KB rows), half of
streaming bandwidth, so the gather is resolved host-side and the device
streams its 6.9 MB shard at full HWDGE rate instead.

On device each core computes all dots on the TensorEngine: delta chunks
(ftr - teachor, computed on DVE) are the matmul weights — loaded only
12x total — and the transposed prototype rows stream through as rhs at
N=512, accumulating [64 samples x 512 slots] all-pairs blocks in PSUM
across the 4 contraction chunks.  A 0/1 mask multiply on DVE keeps each
slot's own sample column; since masked-out entries are exactly 0 and
relu(0 - margin) = 0, the final ScalarE Relu(x - margin) activation with
accumulate sums each block with no per-slot reduce.  Host sums the
8x64x13 partials and divides by B*K.
"""
from contextlib import ExitStack

import numpy as np

import concourse.bass as bass
import concourse.mybir as mybir
from concourse.bacc import Bacc
from concourse.bass_utils import run_bass_kernel_spmd

N_IDS = 100000
FEAT = 512
BATCH = 512
K = 100
MARGIN = 0.03
NCORES = 8
BPC = BATCH // NCORES          # 64
COLS = 52                      # 50 real columns + 2 zero-padded
RCOLS = 50
NCH = FEAT // 128              # 4 contraction chunks
SLOTS = COLS * 128             # 6656 slots
BLK = 512                      # slots per PSUM block
NBLK = SLOTS // BLK            # 13 blocks
PASS0 = 7                      # blocks in pass 0 (PSUM banks 0..6)

f32 = mybir.dt.float32
bf16 = mybir.dt.bfloat16


def _legalize_waits(nc, max_waits=1):
    """This container's walrus rejects instructions carrying more than one
    sync wait.  Hoist extra waits onto standalone InstEventSemaphore ops on
    the same engine queue immediately before the instruction — engine queues
    run in order, so semantics are identical."""
    n = 0
    for f in nc.m.functions:
        for bb in f.blocks:
            insts = list(bb.instructions)
            out = []
            changed = False
            for inst in insts:
                si = inst.sync_info
                waits = list(si.on_wait) if si and si.on_wait else []
                if (
                    len(waits) > max_waits
                    and type(inst).__name__ != "InstEventSemaphore"
                ):
                    for w in waits[:-max_waits]:
                        n += 1
                        ev = mybir.InstEventSemaphore(
                            name=f"hoistw-{n}",
                            ins=[],
                            outs=[],
                            sync_info=mybir.SyncInfo(on_wait=[w], on_update=[]),
                        )
                        ev.engine = inst.engine
                        out.append(ev)
                    si.on_wait = waits[-max_waits:]
                    changed = True
                out.append(inst)
            if changed:
                try:
                    bb.instructions = out
                except Exception:
                    while len(bb.instructions):
                        bb.remove_instruction(bb.instructions[-1])
                    for i in out:
                        bb.add_instruction(i)
    return n


def build_nc():
    nc = Bacc("TRN2")
    rows_d = nc.dram_tensor("rowsPE", [128, NBLK, NCH, BLK], bf16, kind="ExternalInput")
    ftr_d = nc.dram_tensor("ftrT", [128, NCH, BPC], f32, kind="ExternalInput")
    tch_d = nc.dram_tensor("tchT", [128, NCH, BPC], f32, kind="ExternalInput")
    msk_d = nc.dram_tensor("mask", [BPC, BLK], f32, kind="ExternalInput")
    cst_d = nc.dram_tensor("consts", [BPC, 2], f32, kind="ExternalInput")
    out_d = nc.dram_tensor("partial", [BPC, NBLK], f32, kind="ExternalOutput")

    # W load split points (blocks): fine-grained so PE never starves
    LD = [0, 2, 4, 8, NBLK]

    with ExitStack() as ctx:
        block = ctx.enter_context(nc.Block())
        sb = lambda *a: ctx.enter_context(nc.sbuf_tensor(*a))
        sem = lambda n: ctx.enter_context(nc.semaphore(n))
        W = sb("W", [128, NBLK, NCH, BLK], bf16)     # fully resident, 52KB/part
        fT = sb("fT", [128, NCH, BPC], f32)
        tT = sb("tT", [128, NCH, BPC], f32)
        d32 = sb("d32", [128, NCH, BPC], f32)
        dT = sb("dT", [128, NCH, BPC], bf16)
        msk = sb("msk", [BPC, BLK], f32)
        cst = sb("cst", [BPC, 2], f32)
        masked = sb("masked", [BPC, 2, BLK], f32)
        trash = sb("trash", [BPC, BLK], f32)
        part = sb("part", [BPC, NBLK], f32)
        P = [
            ctx.enter_context(nc.psum_tensor(f"P{i}", [BPC, BLK], f32))
            for i in range(8)
        ]
        io_ft = sem("io_ft"); io_msk = sem("io_msk"); io_cst = sem("io_cst")
        io_out = sem("io_out"); gsem = sem("gsem"); dsem = sem("dsem")
        pe_b = sem("pe_b"); vx = sem("vx"); asem = sem("asem")

        nbias = cst[:, 0:1]

        @block.sync
        def _(sp):
            sp.dma_start(fT[:], ftr_d[:]).then_inc(io_ft, 16)
            sp.dma_start(tT[:], tch_d[:]).then_inc(io_ft, 16)
            sp.dma_start(msk[:], msk_d[:]).then_inc(io_msk, 16)
            sp.dma_start(cst[:], cst_d[:]).then_inc(io_cst, 16)
            for li in range(len(LD) - 1):
                sp.dma_start(
                    W[:, LD[li] : LD[li + 1]], rows_d[:, LD[li] : LD[li + 1]]
                ).then_inc(gsem, 16)
            sp.wait_ge(asem, NBLK)
            sp.dma_start(out_d[:], part[:]).then_inc(io_out, 16)
            sp.wait_ge(io_out, 16)

        @block.vector
        def _(v):
            v.wait_ge(io_ft, 32)
            nc.vector.tensor_sub(d32[:], fT[:], tT[:])
            nc.vector.tensor_copy(dT[:], d32[:]).then_inc(dsem, 1)
            v.wait_ge(io_msk, 16)
            for k in range(NBLK):
                bank = k if k < 8 else k - 8
                v.wait_ge(pe_b, k + 1)
                if k >= 2:
                    # masked ring reuse: ACT must have consumed block k-2
                    v.wait_ge(asem, k - 1)
                nc.vector.tensor_tensor(
                    out=masked[:, k % 2, :],
                    in0=P[bank][:],
                    in1=msk[:],
                    op=mybir.AluOpType.mult,
                ).then_inc(vx, 1)

        @block.tensor
        def _(t):
            t.wait_ge(dsem, 1)
            # three passes so extraction overlaps the next pass's matmuls
            for blks in (range(0, 4), range(4, 8), range(8, NBLK)):
                for j in range(NCH):
                    for bk in blks:
                        bank = bk if bk < 8 else bk - 8
                        need = next(
                            i for i in range(1, len(LD)) if bk < LD[i]
                        )
                        if j == 0:
                            t.wait_ge(gsem, 16 * need)
                            if bk >= 8:
                                # bank reuse: block bk-8 must be extracted
                                t.wait_ge(vx, bk - 8 + 1)
                        inst = nc.tensor.matmul(
                            out=P[bank][:],
                            lhsT=dT[:, j, :],
                            rhs=W[:, bk, j, :],
                            start=(j == 0),
                            stop=(j == NCH - 1),
                        )
                        if j == NCH - 1:
                            inst.then_inc(pe_b, 1)

        @block.scalar
        def _(s):
            s.wait_ge(io_cst, 16)
            for k in range(NBLK):
                s.wait_ge(vx, k + 1)
                nc.scalar.activation(
                    out=trash[:],
                    in_=masked[:, k % 2, :],
                    func=mybir.ActivationFunctionType.Relu,
                    bias=nbias,
                    scale=1.0,
                    accum_out=part[:, k : k + 1],
                ).then_inc(asem, 1)

    nc.compile()
    _legalize_waits(nc)
    return nc


def make_in_maps(ftr, teachor_ftr, label, id_prototypes, idH):
    ftr = np.asarray(ftr, dtype=np.float32)
    tch = np.asarray(teachor_ftr, dtype=np.float32)
    label = np.asarray(label).astype(np.int64)
    idH = np.asarray(idH).astype(np.int64)
    protos = np.array(np.asarray(id_prototypes, dtype=np.float32), copy=True)
    protos[label] = tch
    protos16 = protos.astype(mybir.dt.np(bf16))

    neg = idH[label, :K]
    cc = np.arange(RCOLS)
    # mask[b, s] = 1 iff slot s belongs to sample b  (slot = c*128 + p, b = p%64)
    b = np.arange(BPC)[:, None]
    s = np.arange(BLK)[None, :]
    mask = ((s % 128) % BPC == b).astype(np.float32)

    in_maps = []
    for core in range(NCORES):
        sl = slice(core * BPC, (core + 1) * BPC)
        neg_c = neg[sl]
        gidx = np.empty((128, RCOLS), dtype=np.int64)
        gidx[:BPC, :] = neg_c[:, 2 * cc]
        gidx[BPC:, :] = neg_c[:, 2 * cc + 1]
        rows = np.zeros((128, COLS, FEAT), dtype=mybir.dt.np(bf16))
        rows[:, :RCOLS] = protos16[gidx]
        # slot-major: slot = c*128 + p
        slotmat = rows.transpose(1, 0, 2).reshape(SLOTS, FEAT)
        rowsPE = np.ascontiguousarray(
            slotmat.reshape(NBLK, BLK, NCH, 128).transpose(3, 0, 2, 1)
        )  # [p, bk, j, s]

        def tr(x):
            return np.ascontiguousarray(
                x.T.reshape(NCH, 128, BPC).transpose(1, 0, 2)
            )

        consts = np.zeros((BPC, 2), dtype=np.float32)
        consts[:, 0] = -MARGIN
        in_maps.append(
            {
                "rowsPE": rowsPE,
                "ftrT": tr(ftr[sl]),
                "tchT": tr(tch[sl]),
                "mask": mask,
                "consts": consts,
            }
        )
    return in_maps


def finish(results):
    total = np.float64(0.0)
    for r in results:
        total += np.asarray(r["partial"], dtype=np.float64).sum()
    return np.float32(total / (BATCH * K))


_NC_CACHE = {}


def kernel(ftr, teachor_ftr, label, id_prototypes, idH, _trace=False):
    if "nc" not in _NC_CACHE:
        _NC_CACHE["nc"] = build_nc()
    nc = _NC_CACHE["nc"]
    in_maps = make_in_maps(ftr, teachor_ftr, label, id_prototypes, idH)
    res = run_bass_kernel_spmd(nc, in_maps, list(range(NCORES)), trace=_trace)
    out = finish(res.results)
    if _trace:
        return out, res
    return out



# revision 8
# speedup vs baseline: 1.4206x; 1.4206x over previous
"""Trainium2 Bass kernel for nn_CoupleLoss (retrieval_knn).

Reference computation:
    protos = id_prototypes.at[label].set(teachor_ftr)          # scatter
    gi     = protos[idH[label, :K]]                            # [B, K, D] gather
    loss   = mean(relu(einsum('bkd,bd->bk', gi, ftr - teachor_ftr) - MARGIN))

Key identity: smrs - tmrs = gi . (ftr - teachor_ftr), so only one dot per
(b, k) pair is needed against delta = ftr - teachor_ftr.

Distribution (8 cores): data-parallel over the batch (64 samples/core).
The host performs the index routing (applies the tiny teacher scatter and
resolves each core's 6400 = 64*100 prototype row ids) and ships each core
its row shard in compute order, d-major, quantized to fp8e4m3 (measured
final rel err ~1e-3, gate is 2e-2).  On-device row-gather descriptor
generation tops out at ~8 ns/row, so the gather stays host-side and the
device streams its 3.4 MB shard at full HWDGE rate.

Per-core device schedule (v2 — all-fp8, no DVE):
  * SP queue streams the row shard in 5 chunks (prefetch pipeline).
  * ACT queue loads the small tensors, then pre-warms the Relu table.
  * PE runs warmup matmuls (HAM clock ramp), then per 512-slot block:
    two fp8 DoubleRow matmuls contract delta against the rows (all-pairs
    [64 samples x 512 slots] per block), plus one DoubleRow "mask" matmul
    that adds +BIG at each slot's owner row (lhsT = BIG*I64, rhs = 0/1
    tile).  ACT then reads 2-3 PSUM banks per instruction and computes
    relu(x - BIG - margin) with accumulate: non-owner entries fall below
    zero, owner entries reduce to relu(dot - margin).  This removes the
    per-block DVE mask multiply entirely and amortizes ACT fixed costs.
  * Host sums the 8x64x5 partials and divides by B*K.
"""
from contextlib import ExitStack

import numpy as np

import concourse.bass as bass
import concourse.mybir as mybir
from concourse.bacc import Bacc
from concourse.bass_utils import run_bass_kernel_spmd

N_IDS = 100000
FEAT = 512
BATCH = 512
K = 100
MARGIN = 0.03
NCORES = 8
BPC = BATCH // NCORES          # 64
COLS = 52                      # 50 real columns + 2 zero-padded
RCOLS = 50
NCH = FEAT // 128              # 4 contraction chunks (2 DoubleRow pairs)
SLOTS = COLS * 128             # 6656 slots
BLK = 512                      # slots per PSUM block
NBLK = SLOTS // BLK            # 13 blocks
BIG = 224.0                    # mask offset; max|dot| ~ 160, exact in e4m3
NWARM = 24                     # PE warmup matmuls (HAM clock ramp)

# superblocks: (first block, nblocks, psum tensor id, bank0, asem wait)
SBS = [
    (0, 2, 0, 0, 0),
    (2, 2, 0, 2, 0),
    (4, 3, 1, 0, 0),
    (7, 3, 0, 0, 2),    # reuses PA banks 0-2: wait until A1 read done
    (10, 3, 1, 0, 3),   # reuses PB banks 0-2: wait until A2 read done
]
LD = [0, 2, 4, 7, 10, 13]      # W stream chunk boundaries (blocks)
NSB = len(SBS)

f32 = mybir.dt.float32
fp8 = mybir.dt.float8e4
DR = mybir.MatmulPerfMode.DoubleRow


def _legalize_waits(nc, max_waits=1):
    """This container's walrus rejects instructions carrying more than one
    sync wait.  Hoist extra waits onto standalone InstEventSemaphore ops on
    the same engine queue immediately before the instruction — engine queues
    run in order, so semantics are identical."""
    n = 0
    for f in nc.m.functions:
        for bb in f.blocks:
            insts = list(bb.instructions)
            out = []
            changed = False
            for inst in insts:
                si = inst.sync_info
                waits = list(si.on_wait) if si and si.on_wait else []
                if (
                    len(waits) > max_waits
                    and type(inst).__name__ != "InstEventSemaphore"
                ):
                    for w in waits[:-max_waits]:
                        n += 1
                        ev = mybir.InstEventSemaphore(
                            name=f"hoistw-{n}",
                            ins=[],
                            outs=[],
                            sync_info=mybir.SyncInfo(on_wait=[w], on_update=[]),
                        )
                        ev.engine = inst.engine
                        out.append(ev)
                    si.on_wait = waits[-max_waits:]
                    changed = True
                out.append(inst)
            if changed:
                try:
                    bb.instructions = out
                except Exception:
                    while len(bb.instructions):
                        bb.remove_instruction(bb.instructions[-1])
                    for i in out:
                        bb.add_instruction(i)
    return n


def build_nc():
    nc = Bacc("TRN2")
    rows_d = nc.dram_tensor("rowsPE", [128, NBLK, NCH, BLK], fp8, kind="ExternalInput")
    dT_d = nc.dram_tensor("deltaT", [128, NCH, BPC], fp8, kind="ExternalInput")
    tm_d = nc.dram_tensor("maskT", [128, 2, BLK], fp8, kind="ExternalInput")
    id_d = nc.dram_tensor("maskI", [128, 2, BPC], fp8, kind="ExternalInput")
    bias_d = nc.dram_tensor("bias", [BPC, 1], f32, kind="ExternalInput")
    out_d = nc.dram_tensor("partial", [BPC, NSB], f32, kind="ExternalOutput")

    with ExitStack() as ctx:
        block = ctx.enter_context(nc.Block())
        sb = lambda *a: ctx.enter_context(nc.sbuf_tensor(*a))
        sem = lambda n: ctx.enter_context(nc.semaphore(n))
        W = sb("W", [128, NBLK, NCH, BLK], fp8)      # 26 KB/part, fully resident
        dT = sb("dT", [128, NCH, BPC], fp8)
        Tm = sb("Tm", [128, 2, BLK], fp8)            # 0/1 owner tile (k-subtile 1 = 0)
        Im = sb("Im", [128, 2, BPC], fp8)            # BIG * I64   (k-subtile 1 = 0)
        bias = sb("biasS", [BPC, 1], f32)             # -(BIG + MARGIN)
        junk = sb("junk", [128, 2, 256], fp8)        # warmup operands
        trash = sb("trash", [BPC, NBLK, BLK], f32)
        dmy = sb("dmy", [BPC, 1], f32)
        part = sb("part", [BPC, NSB], f32)
        PA = ctx.enter_context(nc.psum_tensor("PA", [BPC, 4, BLK], f32))
        PB = ctx.enter_context(nc.psum_tensor("PB", [BPC, 4, BLK], f32))
        PS = (PA, PB)
        wsem = [sem(f"wsem{c}") for c in range(len(LD) - 1)]
        dsm = [sem(f"dsm{i}") for i in range(4)]
        jsem = sem("jsem")
        pe_sb = sem("pe_sb"); asem = sem("asem"); iosem = sem("iosem")

        nbias = bias[:, 0:1]

        @block.gpsimd
        def _(g):
            nc.gpsimd.memset(junk[:], 1.0).then_inc(jsem, 1)

        @block.sync
        def _(sp):
            for c in range(len(LD) - 1):
                sp.dma_start(
                    W[:, LD[c] : LD[c + 1]], rows_d[:, LD[c] : LD[c + 1]]
                ).then_inc(wsem[c], 16)
            sp.wait_ge(asem, NSB)
            sp.dma_start(out_d[:], part[:]).then_inc(iosem, 16)
            sp.wait_ge(iosem, 16)

        @block.scalar
        def _(s):
            s.dma_start(dT[:], dT_d[:]).then_inc(dsm[0], 16)
            s.dma_start(Tm[:], tm_d[:]).then_inc(dsm[1], 16)
            s.dma_start(Im[:], id_d[:]).then_inc(dsm[2], 16)
            s.dma_start(bias[:], bias_d[:]).then_inc(dsm[3], 16)
            s.wait_ge(dsm[3], 16)
            # dummy activation: pulls ACT_TABLE_LOAD off the critical path
            nc.scalar.activation(
                out=dmy[:, 0:1],
                in_=bias[:, 0:1],
                func=mybir.ActivationFunctionType.Relu,
            )
            for a, (blk0, nb, pi, b0, _) in enumerate(SBS):
                s.wait_ge(pe_sb, a + 1)
                nc.scalar.activation(
                    out=trash[:, blk0 : blk0 + nb, :],
                    in_=PS[pi][:, b0 : b0 + nb, :],
                    func=mybir.ActivationFunctionType.Relu,
                    bias=nbias,
                    scale=1.0,
                    accum_out=part[:, a : a + 1],
                ).then_inc(asem, 1)

        @block.tensor
        def _(t):
            t.wait_ge(jsem, 1)
            for _ in range(NWARM):
                nc.tensor.matmul(
                    out=PB[:, 3, 0:256],
                    lhsT=junk[:, :, 0:BPC],
                    rhs=junk[:],
                    start=True,
                    stop=True,
                    perf_mode=DR,
                )
            for i in range(3):
                t.wait_ge(dsm[i], 16)
            for si, (blk0, nb, pi, b0, aw) in enumerate(SBS):
                t.wait_ge(wsem[si], 16)
                if aw:
                    t.wait_ge(asem, aw)
                P = PS[pi]
                for jp in range(NCH // 2):
                    for i in range(nb):
                        nc.tensor.matmul(
                            out=P[:, b0 + i, :],
                            lhsT=dT[:, 2 * jp : 2 * jp + 2, :],
                            rhs=W[:, blk0 + i, 2 * jp : 2 * jp + 2, :],
                            start=(jp == 0),
                            stop=False,
                            perf_mode=DR,
                        )
                for i in range(nb):
                    inst = nc.tensor.matmul(
                        out=P[:, b0 + i, :],
                        lhsT=Im[:],
                        rhs=Tm[:],
                        start=False,
                        stop=True,
                        perf_mode=DR,
                    )
                inst.then_inc(pe_sb, 1)

    nc.compile()
    _legalize_waits(nc)
    return nc


def make_in_maps(ftr, teachor_ftr, label, id_prototypes, idH):
    np8 = mybir.dt.np(fp8)
    ftr = np.asarray(ftr, dtype=np.float32)
    tch = np.asarray(teachor_ftr, dtype=np.float32)
    label = np.asarray(label).astype(np.int64)
    idH = np.asarray(idH).astype(np.int64)
    protos = np.array(np.asarray(id_prototypes, dtype=np.float32), copy=True)
    protos[label] = tch
    protos8 = protos.astype(np8)
    delta8 = (ftr - tch).astype(np8)

    neg = idH[label, :K]
    cc = np.arange(RCOLS)

    # shared small tensors
    tm = np.zeros((128, 2, BLK), dtype=np8)
    p64 = np.arange(BPC)[:, None]
    s = np.arange(BLK)[None, :]
    tm[:BPC, 0, :] = (s % BPC == p64).astype(np8)
    im = np.zeros((128, 2, BPC), dtype=np8)
    im[:BPC, 0, :][np.arange(BPC), np.arange(BPC)] = np8(BIG)
    bias = np.full((BPC, 1), -(BIG + MARGIN), dtype=np.float32)

    in_maps = []
    for core in range(NCORES):
        sl = slice(core * BPC, (core + 1) * BPC)
        neg_c = neg[sl]
        gidx = np.empty((128, RCOLS), dtype=np.int64)
        gidx[:BPC, :] = neg_c[:, 2 * cc]
        gidx[BPC:, :] = neg_c[:, 2 * cc + 1]
        rows = np.zeros((128, COLS, FEAT), dtype=np8)
        rows[:, :RCOLS] = protos8[gidx]
        # slot-major: slot = c*128 + p ; owner sample = slot % 64
        slotmat = rows.transpose(1, 0, 2).reshape(SLOTS, FEAT)
        rowsPE = np.ascontiguousarray(
            slotmat.reshape(NBLK, BLK, NCH, 128).transpose(3, 0, 2, 1)
        )  # [p, bk, j, s]
        deltaT = np.ascontiguousarray(
            delta8[sl].T.reshape(NCH, 128, BPC).transpose(1, 0, 2)
        )
        in_maps.append(
            {
                "rowsPE": rowsPE,
                "deltaT": deltaT,
                "maskT": tm,
                "maskI": im,
                "bias": bias,
            }
        )
    return in_maps


def finish(results):
    total = np.float64(0.0)
    for r in results:
        total += np.asarray(r["partial"], dtype=np.float64).sum()
    return np.float32(total / (BATCH * K))


_NC_CACHE = {}


def kernel(ftr, teachor_ftr, label, id_prototypes, idH, _trace=False):
    if "nc" not in _NC_CACHE:
        _NC_CACHE["nc"] = build_nc()
    nc = _NC_CACHE["nc"]
    in_maps = make_in_maps(ftr, teachor_ftr, label, id_prototypes, idH)
    res = run_bass_kernel_spmd(nc, in_maps, list(range(NCORES)), trace=_trace)
    out = finish(res.results)
    if _trace:
        return out, res
    return out
